# revision 32
# baseline (speedup 1.0000x reference)
"""AnomalyTransformer forward on 8 TRN2 NeuronCores — data parallel over batch.

Self-contained: hardcodes shapes (B=8, L=512, ENC_IN=38, D=512, H=8, DFF=512,
NLAYERS=3) and shards batch b -> core b.  Returns the reference pytree
(out, (series0..2), (prior0..2), (sigma0..2)) as float32 numpy arrays.
"""

import math
import os
import sys

import numpy as np

for _p in ("/opt/trn_rl_repo", "/root/.axon_site/_ro/trn_rl_repo"):
    if os.path.isdir(_p) and _p not in sys.path:
        sys.path.insert(0, _p)
        break

B, L, ENC_IN, C_OUT, D, H, DFF, NLAYERS = 8, 512, 38, 38, 512, 8, 512, 3
P = 128
NCH = D // P          # 4 chunks of 128 along any 512 dim
DH = D // H           # 64 head dim
LN3 = math.log(3.0)
NEG_HALF_LN_2PI = -0.5 * math.log(2.0 * math.pi)
EPS_LN = 1e-5

# Output HBM dtype for the big [H,L,L] tensors: bfloat16 halves the DMA-out
# traffic (~3e-3 relative error on those outputs); set to 0 for full fp32.
OUT_BF16 = os.environ.get("KERNEL_OUT_BF16", "1") == "1"

_CACHE = {}


def _patch_act_tables():
    # The act-table-load inserter greedily picks the first set containing a
    # function, so alternating Exp/Ln thrash-loads two different sets.  Filter
    # the map it sees so Exp and Ln only resolve to the combined
    # natural_log_exp_and_others set (which really contains both).
    import concourse.bacc as bacc
    import concourse.hw_specs as hw_specs
    import concourse.mybir as mybir

    if getattr(bacc, "_act_tables_patched", False):
        return
    AF = mybir.ActivationFunctionType
    orig = bacc.get_activation_tables

    def patched(arch):
        tables = orig(arch)
        for name, fns in tables.items():
            if name != "natural_log_exp_and_others":
                fns.discard(AF.Exp)
                fns.discard(AF.Ln)
        return tables

    bacc.get_activation_tables = patched
    bacc._act_tables_patched = True


def _build(gates):
    import concourse.bacc as bacc
    import concourse.mybir as mybir
    import concourse.tile as tile
    from concourse.tile_rust import add_dep_helper

    _patch_act_tables()

    dt = mybir.dt
    f32 = dt.float32
    bf16 = dt.bfloat16
    obf = bf16 if OUT_BF16 else f32
    AF = mybir.ActivationFunctionType
    ALU = mybir.AluOpType

    nc = bacc.Bacc(trn_type="TRN2")

    # ---------------- DRAM parameters ----------------
    d_xcatT = nc.dram_tensor("xcatT", [3 * ENC_IN, L], bf16, kind="ExternalInput")
    d_wflat = nc.dram_tensor("wflat", [3 * ENC_IN, D], bf16, kind="ExternalInput")
    d_pe = nc.dram_tensor("pe", [L, D], f32, kind="ExternalInput")
    d_dist2 = nc.dram_tensor("dist2", [L, L], f32, kind="ExternalInput")
    d_ident = nc.dram_tensor("ident", [P, P], f32, kind="ExternalInput")

    d_w = []
    for i in range(NLAYERS):
        lw = {}
        for nmw in ("Wq", "Wk", "Wv", "Wo", "W1", "W2"):
            lw[nmw] = nc.dram_tensor(f"{nmw}{i}", [D, D], bf16, kind="ExternalInput")
        lw["Wsig"] = nc.dram_tensor(f"Wsig{i}", [D, H], f32, kind="ExternalInput")
        lw["bq"] = nc.dram_tensor(f"bq{i}", [P, NCH], f32, kind="ExternalInput")
        lw["bk"] = nc.dram_tensor(f"bk{i}", [P, NCH], f32, kind="ExternalInput")
        lw["bf1"] = nc.dram_tensor(f"bf1{i}", [P, NCH], f32, kind="ExternalInput")
        lw["bv_bc"] = nc.dram_tensor(f"bv_bc{i}", [P, D], f32, kind="ExternalInput")
        lw["bsig_bc"] = nc.dram_tensor(f"bsig_bc{i}", [P, H], f32, kind="ExternalInput")
        for nmo in ("bo_bc", "bf2_bc"):
            if gates[nmo[:-3]]:
                lw[nmo] = nc.dram_tensor(f"{nmo}{i}", [P, D], f32, kind="ExternalInput")
        if gates["ln1"]:
            lw["g1_bc"] = nc.dram_tensor(f"g1_bc{i}", [P, D], f32, kind="ExternalInput")
            lw["b1_bc"] = nc.dram_tensor(f"b1_bc{i}", [P, D], f32, kind="ExternalInput")
        if gates["ln2"]:
            lw["g2_bc"] = nc.dram_tensor(f"g2_bc{i}", [P, D], f32, kind="ExternalInput")
            lw["b2_bc"] = nc.dram_tensor(f"b2_bc{i}", [P, D], f32, kind="ExternalInput")
        d_w.append(lw)
    d_wp = nc.dram_tensor("Wp", [D, C_OUT], bf16, kind="ExternalInput")
    if gates["lnf"]:
        d_gn = nc.dram_tensor("gn_bc", [P, D], f32, kind="ExternalInput")
        d_bn = nc.dram_tensor("bn_bc", [P, D], f32, kind="ExternalInput")
    if gates["bp"]:
        d_bp = nc.dram_tensor("bp_bc", [P, C_OUT], f32, kind="ExternalInput")

    d_out = nc.dram_tensor("out", [L, C_OUT], f32, kind="ExternalOutput")
    d_series = [
        nc.dram_tensor(f"series{i}", [H, L, L], obf, kind="ExternalOutput")
        for i in range(NLAYERS)
    ]
    d_prior = [
        nc.dram_tensor(f"prior{i}", [H, L, L], obf, kind="ExternalOutput")
        for i in range(NLAYERS)
    ]
    d_sigma = [
        nc.dram_tensor(f"sigma{i}", [H, L, L], obf, kind="ExternalOutput")
        for i in range(NLAYERS)
    ]

    def chunked(dram):  # [512, N] dram -> [128, 4, N] AP
        return dram[:, :].rearrange("(c p) n -> p c n", p=P)

    with tile.TileContext(nc) as tc:
        with (
            tc.tile_pool(name="const", bufs=1) as cpool,
            tc.tile_pool(name="wts", bufs=1) as wpool,
            tc.tile_pool(name="st1", bufs=1) as spool1,
            tc.tile_pool(name="st2", bufs=2) as spool2,
            tc.tile_pool(name="tmat", bufs=2) as tpool,
            tc.tile_pool(name="proj", bufs=1) as ppool,
            tc.tile_pool(name="ser", bufs=5 if OUT_BF16 else 3) as serpool,
            tc.tile_pool(name="serT", bufs=4) as serTpool,
            tc.tile_pool(name="outs", bufs=2) as opool,
            tc.tile_pool(name="small", bufs=2) as smpool,
            tc.tile_pool(name="ps512", bufs=4, space="PSUM") as ps512,
            tc.tile_pool(name="psT", bufs=3, space="PSUM") as psT,
            tc.tile_pool(name="psAT", bufs=1, space="PSUM") as psAT,
        ):
            ident = cpool.tile([P, P], f32)
            ones = cpool.tile([P, D], f32)
            nc.vector.memset(ones[:], 1.0)
            ones_bf = cpool.tile([P, D], bf16)
            nc.vector.memset(ones_bf[:], 1.0)
            eps_t = cpool.tile([P, 1], f32)
            nc.vector.memset(eps_t[:], EPS_LN)
            dist2 = cpool.tile([P, NCH, L], f32)
            wp_sb = cpool.tile([P, NCH, C_OUT], bf16)
            out_sb = cpool.tile([P, NCH, C_OUT], f32)
            ident_bf = cpool.tile([P, P], bf16)
            if gates["lnf"]:
                gn_sb = cpool.tile([P, D], f32)
                nc.sync.dma_start(gn_sb[:], d_gn[:, :])
                bn_sb = cpool.tile([P, D], f32)
                nc.sync.dma_start(bn_sb[:], d_bn[:, :])
            if gates["bp"]:
                bp_sb = cpool.tile([P, C_OUT], f32)
                nc.sync.dma_start(bp_sb[:], d_bp[:, :])

            def mm(out, lhsT, rhs, start, stop):
                nc.tensor.matmul(out, lhsT, rhs, start=start, stop=stop)

            # PE transpose of a [128, 4, 512] natural tile -> [128, 4, 512]
            # transposed tile (chunk index swaps roles).
            def transpose_512(dst_tile, src_tile, dst2=None, on_scalar=False):
                for n in range(NCH):
                    ps = psT.tile([P, L], f32, tag="psT")
                    for m in range(NCH):
                        nc.tensor.transpose(
                            ps[:, m * P : (m + 1) * P],
                            src_tile[:, m, n * P : (n + 1) * P],
                            ident[:],
                        )
                    if on_scalar:
                        nc.scalar.copy(dst_tile[:, n, :], ps[:])
                    else:
                        nc.vector.tensor_copy(dst_tile[:, n, :], ps[:])
                    if dst2 is not None:
                        nc.scalar.copy(dst2[:, n, :], ps[:])

            def layernorm(dst_tile, src_tile, g_bc, b_bc):
                mv = smpool.tile([P, NCH, 2], f32, tag="mv", name="mv")
                for c in range(NCH):
                    st6 = smpool.tile([P, 6], f32, tag="st6", name="st6")
                    nc.vector.bn_stats(st6[:], src_tile[:, c, :])
                    nc.vector.bn_aggr(mv[:, c, :], st6[:])
                lnv = smpool.tile([P, NCH], f32, tag="lnv", name="lnv")
                nc.scalar.activation(lnv[:], mv[:, :, 1], AF.Ln, bias=eps_t[:])
                rstd = smpool.tile([P, NCH], f32, tag="rstd", name="rstd")
                nc.scalar.activation(rstd[:], lnv[:], AF.Exp, scale=-0.5)
                for c in range(NCH):
                    nc.vector.tensor_scalar(
                        dst_tile[:, c, :],
                        src_tile[:, c, :],
                        mv[:, c, 0:1],
                        rstd[:, c : c + 1],
                        op0=ALU.subtract,
                        op1=ALU.mult,
                    )
                    if g_bc is not None:
                        nc.vector.scalar_tensor_tensor(
                            dst_tile[:, c, :], dst_tile[:, c, :], 1.0, g_bc[:],
                            op0=ALU.mult, op1=ALU.mult,
                        )
                    if b_bc is not None:
                        nc.vector.scalar_tensor_tensor(
                            dst_tile[:, c, :], dst_tile[:, c, :], 1.0, b_bc[:],
                            op0=ALU.mult, op1=ALU.add,
                        )

            # ---------------- embedding ----------------
            h_nat = spool1.tile([P, NCH, D], f32, tag="h")
            with tc.tile_pool(name="embed", bufs=1) as epool:
                xcatT = epool.tile([3 * ENC_IN, L], bf16)
                nc.sync.dma_start(xcatT[:], d_xcatT[:, :])
                wflat = epool.tile([3 * ENC_IN, D], bf16)
                nc.sync.dma_start(wflat[:], d_wflat[:, :])
                pe_sb = epool.tile([P, NCH, D], f32)
                nc.sync.dma_start(pe_sb[:], chunked(d_pe))
                for m in range(NCH):
                    ps = ps512.tile([P, D], f32, tag="ps512")
                    mm(ps[:], xcatT[:, m * P : (m + 1) * P], wflat[:], True, True)
                    nc.vector.scalar_tensor_tensor(
                        h_nat[:, m, :], ps[:], 1.0, pe_sb[:, m, :],
                        op0=ALU.mult, op1=ALU.add,
                    )

            nc.sync.dma_start(ident[:], d_ident[:, :])
            nc.vector.tensor_copy(ident_bf[:], ident[:])
            warm_ps = psT.tile([P, L], f32, tag="psT", name="warm_ps")
            for _w in range(24):
                nc.tensor.matmul(
                    warm_ps[:, 0:P], ident[:], ident[:],
                    start=True, stop=True,
                )
            nc.sync.dma_start(dist2[:], chunked(d_dist2))
            nc.sync.dma_start(wp_sb[:], d_wp[:, :].rearrange("(c p) n -> p c n", p=P))

            # ---------------- layers ----------------
            for li in range(NLAYERS):
                lw = d_w[li]
                wq = wpool.tile([P, NCH, D], bf16, tag="wq")
                nc.sync.dma_start(wq[:], chunked(lw["Wq"]))
                wk = wpool.tile([P, NCH, D], bf16, tag="wk")
                nc.sync.dma_start(wk[:], chunked(lw["Wk"]))
                wv = wpool.tile([P, NCH, D], bf16, tag="wv")
                nc.sync.dma_start(wv[:], chunked(lw["Wv"]))
                wo = wpool.tile([P, NCH, D], bf16, tag="wo")
                nc.sync.dma_start(wo[:], chunked(lw["Wo"]))
                w1 = wpool.tile([P, NCH, D], bf16, tag="w1")
                nc.sync.dma_start(w1[:], chunked(lw["W1"]))
                w2 = wpool.tile([P, NCH, D], bf16, tag="w2")
                nc.sync.dma_start(w2[:], chunked(lw["W2"]))
                wsig = wpool.tile([P, NCH, H], f32, tag="wsig")
                nc.sync.dma_start(wsig[:], chunked(lw["Wsig"]))
                bq_sb = wpool.tile([P, NCH], f32, tag="bq")
                nc.sync.dma_start(bq_sb[:], lw["bq"][:, :])
                bk_sb = wpool.tile([P, NCH], f32, tag="bk")
                nc.sync.dma_start(bk_sb[:], lw["bk"][:, :])
                bf1_sb = wpool.tile([P, NCH], f32, tag="bf1")
                nc.sync.dma_start(bf1_sb[:], lw["bf1"][:, :])
                bv_sb = wpool.tile([P, D], f32, tag="bv")
                nc.sync.dma_start(bv_sb[:], lw["bv_bc"][:, :])
                bsig_sb = wpool.tile([P, H], f32, tag="bsig")
                nc.sync.dma_start(bsig_sb[:], lw["bsig_bc"][:, :])
                opt = {}
                for nmo in ("bo_bc", "bf2_bc", "g1_bc", "b1_bc", "g2_bc", "b2_bc"):
                    if nmo in lw:
                        t = wpool.tile([P, D], f32, tag=nmo)
                        nc.sync.dma_start(t[:], lw[nmo][:, :])
                        opt[nmo] = t

                # -- transpose h -> hT --
                hT = tpool.tile([P, NCH, D], bf16, tag="tT")
                hT32 = tpool.tile([P, NCH, D], f32, tag="hT32")
                transpose_512(hT, h_nat, dst2=hT32)

                # -- projections --
                qT = ppool.tile([P, NCH, L], bf16, tag="qT")
                kT = ppool.tile([P, NCH, L], bf16, tag="kT")
                v_sb = ppool.tile([P, NCH, D], bf16, tag="v")
                for c in range(NCH):
                    ps = ps512.tile([P, L], f32, tag="ps512")
                    for kc in range(NCH):
                        mm(ps[:], wq[:, kc, c * P : (c + 1) * P], hT[:, kc, :],
                           kc == 0, kc == NCH - 1)
                    nc.scalar.activation(
                        qT[:, c, :], ps[:], AF.Identity, bias=bq_sb[:, c : c + 1]
                    )
                for c in range(NCH):
                    ps = ps512.tile([P, L], f32, tag="ps512")
                    for kc in range(NCH):
                        mm(ps[:], wk[:, kc, c * P : (c + 1) * P], hT[:, kc, :],
                           kc == 0, kc == NCH - 1)
                    nc.scalar.activation(
                        kT[:, c, :], ps[:], AF.Identity, bias=bk_sb[:, c : c + 1]
                    )
                for c in range(NCH):
                    ps = ps512.tile([P, D], f32, tag="ps512")
                    for kc in range(NCH):
                        mm(ps[:], hT[:, kc, c * P : (c + 1) * P], wv[:, kc, :],
                           kc == 0, kc == NCH - 1)
                    nc.vector.scalar_tensor_tensor(
                        v_sb[:, c, :], ps[:], 1.0, bv_sb[:],
                        op0=ALU.mult, op1=ALU.add,
                    )

                # -- sigma scalar path (batched across chunks) --
                s_val = smpool.tile([P, NCH, H], f32, tag="s_val")
                a_val = smpool.tile([P, NCH, H], f32, tag="a_val")
                lnc = smpool.tile([P, NCH, H], f32, tag="lnc")
                sg_u = smpool.tile([P, NCH, H], f32, tag="sg_u")
                for c in range(NCH):
                    ps8 = psAT.tile([P, H], f32, tag="psAT")
                    for kc in range(NCH):
                        mm(ps8[:], hT32[:, kc, c * P : (c + 1) * P], wsig[:, kc, :],
                           kc == 0, kc == NCH - 1)
                    nc.vector.scalar_tensor_tensor(
                        sg_u[:, c, :], ps8[:], 1.0, bsig_sb[:],
                        op0=ALU.mult, op1=ALU.add,
                    )
                NH = NCH * H
                # sigmoid(5u) = 1 / (1 + exp(-5u))
                t0 = smpool.tile([P, NH], f32, tag="t0")
                nc.scalar.activation(t0[:], sg_u[:], AF.Exp, scale=-5.0)
                nc.vector.tensor_scalar(t0[:], t0[:], 1.0, None, op0=ALU.add)
                t1 = smpool.tile([P, NH], f32, tag="t1")
                nc.vector.reciprocal(t1[:], t0[:])
                nc.vector.tensor_scalar(t1[:], t1[:], 1e-5, None, op0=ALU.add)
                # s = 3^t1 - 1 = exp(t1*ln3) - 1
                t2 = smpool.tile([P, NH], f32, tag="t2")
                nc.scalar.activation(t2[:], t1[:], AF.Exp, scale=LN3)
                nc.vector.tensor_scalar(
                    s_val[:].rearrange("p c h -> p (c h)"), t2[:], 1.0, None,
                    op0=ALU.subtract,
                )
                # a = -0.5 / s^2 ; lnc = -0.5*ln(2pi) - ln(s)
                t3 = smpool.tile([P, NH], f32, tag="t3")
                nc.vector.tensor_tensor(
                    t3[:], s_val[:].rearrange("p c h -> p (c h)"),
                    s_val[:].rearrange("p c h -> p (c h)"), op=ALU.mult,
                )
                t4 = smpool.tile([P, NH], f32, tag="t4")
                nc.vector.reciprocal(t4[:], t3[:])
                nc.vector.tensor_scalar(
                    a_val[:].rearrange("p c h -> p (c h)"), t4[:], -0.5, None,
                    op0=ALU.mult,
                )
                t5 = smpool.tile([P, NH], f32, tag="t5")
                nc.scalar.activation(
                    t5[:], s_val[:].rearrange("p c h -> p (c h)"), AF.Ln
                )
                nc.vector.tensor_scalar(
                    lnc[:].rearrange("p c h -> p (c h)"), t5[:], -1.0,
                    NEG_HALF_LN_2PI, op0=ALU.mult, op1=ALU.add,
                )

                # -- attention --
                rowsums = smpool.tile([P, NCH, H], f32, tag="rowsums")
                recips = smpool.tile([P, NCH, H], f32, tag="recips")
                last_prior = None
                attnT_sb = tpool.tile([P, NCH, D], bf16, tag="tT")
                psat_tiles = [None] * NCH

                def prior_sigma(m, half):
                    pr = opool.tile([P, 4, L], obf, tag="prior", name="pr")
                    sg = opool.tile([P, 4, L], obf, tag="sigma", name="sg")
                    lp = None
                    for h4 in range(4):
                        h = half * 4 + h4
                        lp = nc.scalar.activation(
                            pr[:, h4, :], dist2[:, m, :], AF.Exp,
                            scale=a_val[:, m, h : h + 1],
                            bias=lnc[:, m, h : h + 1],
                        )
                        nc.vector.tensor_scalar(
                            sg[:, h4, :], ones_bf[:, 0:L],
                            s_val[:, m, h : h + 1], None, op0=ALU.mult,
                        )
                    dstp = d_prior[li][
                        half * 4 : half * 4 + 4, m * P : (m + 1) * P, :
                    ].rearrange("h l s -> l h s")
                    nc.sync.dma_start(dstp, pr[:])
                    dsts = d_sigma[li][
                        half * 4 : half * 4 + 4, m * P : (m + 1) * P, :
                    ].rearrange("h l s -> l h s")
                    nc.sync.dma_start(dsts, sg[:])
                    return lp

                def head_transpose_attn(half, h4, nrm_tiles):
                    h = half * 4 + h4
                    serT = serTpool.tile([P, NCH, L], obf, tag="serT", name="serT")
                    for sc in range(NCH):
                        pst = psT.tile([P, L], obf, tag="psT", name="psTb")
                        for m in range(NCH):
                            nc.tensor.transpose(
                                pst[:, m * P : (m + 1) * P],
                                nrm_tiles[m][:, h4, sc * P : (sc + 1) * P],
                                ident_bf[:] if OUT_BF16 else ident[:],
                            )
                        nc.vector.tensor_copy(serT[:, sc, :], pst[:])
                    # attnT[d, l] += v[s, d] @ serT[s, l]; two heads pack a
                    # [128, 512] psum tile (column groups 0-63 / 64-127)
                    t_idx = h // 2
                    doff = (h % 2) * DH
                    if h % 2 == 0:
                        psat_tiles[t_idx] = psAT.tile(
                            [P, L], f32, tag="psAT", name="psat"
                        )
                    psat_cur = psat_tiles[t_idx]
                    for sc in range(NCH):
                        nc.tensor.matmul(
                            psat_cur[doff : doff + DH, :],
                            v_sb[:, sc, h * DH : (h + 1) * DH],
                            serT[:, sc, :],
                            start=(sc == 0), stop=(sc == NCH - 1),
                            tile_position=(0, doff),
                        )
                    if h % 2 == 1:
                        nc.vector.tensor_copy(attnT_sb[:, t_idx, :], psat_cur[:])

                for half in range(2):
                    ser_tiles = []
                    nrm_tiles = []
                    for m in range(NCH):
                        ser_tiles.append(
                            serpool.tile([P, 4, L], obf, tag="ser", name=f"ser{m}")
                        )
                        nrm_tiles.append(
                            serpool.tile([P, 4, L], obf, tag="nrm", name=f"nrm{m}")
                        )
                    prev_h4 = None
                    for h4 in range(4):
                        h = half * 4 + h4
                        hc, off = h // 2, (h % 2) * DH
                        for m in range(NCH):
                            ps = ps512.tile([P, L], f32, tag="ps512")
                            mm(ps[:], qT[off : off + DH, hc, m * P : (m + 1) * P],
                               kT[off : off + DH, hc, :], True, True)
                            nc.scalar.activation(
                                ser_tiles[m][:, h4, :], ps[:], AF.Exp, scale=0.125,
                                accum_out=rowsums[:, m, h : h + 1],
                            )
                        nc.vector.reciprocal(
                            recips[:, :, h : h + 1], rowsums[:, :, h : h + 1]
                        )
                        for m in range(NCH):
                            nc.vector.tensor_scalar(
                                nrm_tiles[m][:, h4, :], ser_tiles[m][:, h4, :],
                                recips[:, m, h : h + 1], None, op0=ALU.mult,
                            )
                        # pipeline: previous head's transpose+attn trace AFTER
                        # this head's scores/exp so PE keeps ACT fed
                        if prev_h4 is not None:
                            head_transpose_attn(half, prev_h4, nrm_tiles)
                        prev_h4 = h4
                    for m in range(NCH):
                        dst = d_series[li][
                            half * 4 : half * 4 + 4, m * P : (m + 1) * P, :
                        ].rearrange("h l s -> l h s")
                        nc.sync.dma_start(dst, nrm_tiles[m][:])
                    head_transpose_attn(half, prev_h4, nrm_tiles)
                    for m in range(NCH):
                        last_prior = prior_sigma(m, half)


                # -- output projection + residual + LN1 --
                attnT = attnT_sb
                h2 = spool2.tile([P, NCH, D], f32, tag="h2")
                for m in range(NCH):
                    ps = ps512.tile([P, D], f32, tag="ps512")
                    for kc in range(NCH):
                        mm(ps[:], attnT[:, kc, m * P : (m + 1) * P], wo[:, kc, :],
                           kc == 0, kc == NCH - 1)
                    nc.vector.scalar_tensor_tensor(
                        h2[:, m, :], ps[:], 1.0, h_nat[:, m, :],
                        op0=ALU.mult, op1=ALU.add,
                    )
                    if "bo_bc" in opt:
                        nc.vector.scalar_tensor_tensor(
                            h2[:, m, :], h2[:, m, :], 1.0, opt["bo_bc"][:],
                            op0=ALU.mult, op1=ALU.add,
                        )
                x1 = spool1.tile([P, NCH, D], f32, tag="x1")
                layernorm(x1, h2, opt.get("g1_bc"), opt.get("b1_bc"))

                # -- FFN --
                x1T = tpool.tile([P, NCH, D], bf16, tag="tT")
                transpose_512(x1T, x1)
                y1g = ppool.tile([P, NCH, L], bf16, tag="qT")
                for c in range(NCH):
                    ps = ps512.tile([P, L], f32, tag="ps512")
                    for kc in range(NCH):
                        mm(ps[:], w1[:, kc, c * P : (c + 1) * P], x1T[:, kc, :],
                           kc == 0, kc == NCH - 1)
                    g_inst = nc.scalar.activation(
                        y1g[:, c, :], ps[:], AF.Gelu, bias=bf1_sb[:, c : c + 1]
                    )
                    if last_prior is not None:
                        add_dep_helper(
                            g_inst.ins, last_prior.ins, sync=False,
                            reason="group ACT table sets: priors before gelu",
                        )
                h3 = spool2.tile([P, NCH, D], f32, tag="h2")
                for m in range(NCH):
                    ps = ps512.tile([P, D], f32, tag="ps512")
                    for kc in range(NCH):
                        mm(ps[:], y1g[:, kc, m * P : (m + 1) * P], w2[:, kc, :],
                           kc == 0, kc == NCH - 1)
                    nc.vector.scalar_tensor_tensor(
                        h3[:, m, :], ps[:], 1.0, x1[:, m, :],
                        op0=ALU.mult, op1=ALU.add,
                    )
                    if "bf2_bc" in opt:
                        nc.vector.scalar_tensor_tensor(
                            h3[:, m, :], h3[:, m, :], 1.0, opt["bf2_bc"][:],
                            op0=ALU.mult, op1=ALU.add,
                        )
                h_nat = spool1.tile([P, NCH, D], f32, tag="h")
                layernorm(h_nat, h3, opt.get("g2_bc"), opt.get("b2_bc"))

            # ---------------- final norm + projection ----------------
            hf = spool1.tile([P, NCH, D], f32, tag="x1")
            layernorm(
                hf, h_nat,
                gn_sb if gates["lnf"] else None,
                bn_sb if gates["lnf"] else None,
            )
            hfT = tpool.tile([P, NCH, D], bf16, tag="tT")
            transpose_512(hfT, hf)
            for m in range(NCH):
                ps = psAT.tile([P, C_OUT], f32, tag="psAT")
                for kc in range(NCH):
                    mm(ps[:], hfT[:, kc, m * P : (m + 1) * P], wp_sb[:, kc, :],
                       kc == 0, kc == NCH - 1)
                if gates["bp"]:
                    nc.vector.scalar_tensor_tensor(
                        out_sb[:, m, :], ps[:], 1.0, bp_sb[:],
                        op0=ALU.mult, op1=ALU.add,
                    )
                else:
                    nc.vector.tensor_copy(out_sb[:, m, :], ps[:])
            nc.sync.dma_start(
                d_out[:, :].rearrange("(c p) n -> p c n", p=P), out_sb[:]
            )

    nc.finalize()
    return nc


def _np(a):
    return np.asarray(a, dtype=np.float32)


def _bf(a):
    import ml_dtypes

    return np.asarray(a, dtype=np.float32).astype(ml_dtypes.bfloat16)


def _prep_inputs(x, params):
    """Host-side layout prep: pure indexing/broadcast of inputs + constants."""
    x = _np(x)
    layers = params["layers"]

    idx = np.arange(L, dtype=np.float32)
    dist2 = np.square(np.abs(idx[:, None] - idx[None, :])).astype(np.float32)
    pos = idx[:, None]
    div = np.exp(
        np.arange(0, D, 2, dtype=np.float32) * (-math.log(10000.0) / D)
    ).astype(np.float32)
    pe = np.zeros((L, D), np.float32)
    pe[:, 0::2] = np.sin(pos * div)
    pe[:, 1::2] = np.cos(pos * div)
    ident = np.eye(P, dtype=np.float32)
    wflat = _np(params["tok_w"]).reshape(3 * ENC_IN, D)

    def percol(b):  # [512] -> [128, 4] per-partition layout
        return np.ascontiguousarray(_np(b).reshape(NCH, P).T)

    def bc(v, n):
        return np.ascontiguousarray(np.broadcast_to(_np(v), (P, n)))

    common = {
        "wflat": np.ascontiguousarray(_bf(wflat)),
        "pe": pe,
        "dist2": dist2,
        "ident": ident,
        "Wp": _bf(params["Wp"]),
    }
    gates = {}
    gates["bp"] = bool(np.any(_np(params["bp"]) != 0))
    if gates["bp"]:
        common["bp_bc"] = bc(params["bp"], C_OUT)
    gates["lnf"] = bool(
        np.any(_np(params["norm_g"]) != 1) or np.any(_np(params["norm_b"]) != 0)
    )
    if gates["lnf"]:
        common["gn_bc"] = bc(params["norm_g"], D)
        common["bn_bc"] = bc(params["norm_b"], D)
    gates["bo"] = any(np.any(_np(p["bo"]) != 0) for p in layers)
    gates["bf2"] = any(np.any(_np(p["bf2"]) != 0) for p in layers)
    gates["ln1"] = any(
        np.any(_np(p["g1"]) != 1) or np.any(_np(p["b1"]) != 0) for p in layers
    )
    gates["ln2"] = any(
        np.any(_np(p["g2"]) != 1) or np.any(_np(p["b2"]) != 0) for p in layers
    )
    for i, p in enumerate(layers):
        for nmw in ("Wq", "Wk", "Wv", "Wo", "W1", "W2"):
            common[f"{nmw}{i}"] = _bf(p[nmw])
        common[f"Wsig{i}"] = _np(p["Wsig"])
        common[f"bq{i}"] = percol(p["bq"])
        common[f"bk{i}"] = percol(p["bk"])
        common[f"bf1{i}"] = percol(p["bf1"])
        common[f"bv_bc{i}"] = bc(p["bv"], D)
        common[f"bsig_bc{i}"] = bc(p["bsig"], H)
        if gates["bo"]:
            common[f"bo_bc{i}"] = bc(p["bo"], D)
        if gates["bf2"]:
            common[f"bf2_bc{i}"] = bc(p["bf2"], D)
        if gates["ln1"]:
            common[f"g1_bc{i}"] = bc(p["g1"], D)
            common[f"b1_bc{i}"] = bc(p["b1"], D)
        if gates["ln2"]:
            common[f"g2_bc{i}"] = bc(p["g2"], D)
            common[f"b2_bc{i}"] = bc(p["b2"], D)

    in_maps = []
    for b in range(B):
        xb = x[b]
        xm1 = np.roll(xb, 1, axis=0)
        xp1 = np.roll(xb, -1, axis=0)
        xcat = np.concatenate([xm1, xb, xp1], axis=1)  # [512, 114]
        m = dict(common)
        m["xcatT"] = np.ascontiguousarray(_bf(xcat.T))
        in_maps.append(m)
    return in_maps, gates


def kernel(x, params, _trace=False):
    from concourse.bass_utils import run_bass_kernel_spmd

    in_maps, gates = _prep_inputs(x, params)
    key = tuple(sorted(gates.items()))
    if key not in _CACHE:
        _CACHE[key] = _build(gates)
    nc = _CACHE[key]

    res = run_bass_kernel_spmd(nc, in_maps, core_ids=list(range(B)), trace=_trace)

    def f32out(name):
        return np.stack(
            [np.asarray(res.results[b][name], np.float32) for b in range(B)], axis=0
        )

    out = f32out("out")
    series = tuple(f32out(f"series{i}") for i in range(NLAYERS))
    prior = tuple(f32out(f"prior{i}") for i in range(NLAYERS))
    sigma = tuple(f32out(f"sigma{i}") for i in range(NLAYERS))
    ret = (out, series, prior, sigma)
    if _trace:
        return ret, res
    return ret


# revision 33
# speedup vs baseline: 1.0171x; 1.0171x over previous
"""AnomalyTransformer forward on 8 TRN2 NeuronCores — data parallel over batch.

Self-contained: hardcodes shapes (B=8, L=512, ENC_IN=38, D=512, H=8, DFF=512,
NLAYERS=3) and shards batch b -> core b.  Returns the reference pytree
(out, (series0..2), (prior0..2), (sigma0..2)) as float32 numpy arrays.
"""

import math
import os
import sys

import numpy as np

for _p in ("/opt/trn_rl_repo", "/root/.axon_site/_ro/trn_rl_repo"):
    if os.path.isdir(_p) and _p not in sys.path:
        sys.path.insert(0, _p)
        break

B, L, ENC_IN, C_OUT, D, H, DFF, NLAYERS = 8, 512, 38, 38, 512, 8, 512, 3
P = 128
NCH = D // P          # 4 chunks of 128 along any 512 dim
DH = D // H           # 64 head dim
LN3 = math.log(3.0)
NEG_HALF_LN_2PI = -0.5 * math.log(2.0 * math.pi)
EPS_LN = 1e-5

# Output HBM dtype for the big [H,L,L] tensors: bfloat16 halves the DMA-out
# traffic (~3e-3 relative error on those outputs); set to 0 for full fp32.
OUT_BF16 = os.environ.get("KERNEL_OUT_BF16", "1") == "1"

_CACHE = {}


def _patch_act_tables():
    # The act-table-load inserter greedily picks the first set containing a
    # function, so alternating Exp/Ln thrash-loads two different sets.  Filter
    # the map it sees so Exp and Ln only resolve to the combined
    # natural_log_exp_and_others set (which really contains both).
    import concourse.bacc as bacc
    import concourse.hw_specs as hw_specs
    import concourse.mybir as mybir

    if getattr(bacc, "_act_tables_patched", False):
        return
    AF = mybir.ActivationFunctionType
    orig = bacc.get_activation_tables

    def patched(arch):
        tables = orig(arch)
        for name, fns in tables.items():
            if name != "natural_log_exp_and_others":
                fns.discard(AF.Exp)
                fns.discard(AF.Ln)
        return tables

    bacc.get_activation_tables = patched
    bacc._act_tables_patched = True


def _build(gates):
    import concourse.bacc as bacc
    import concourse.mybir as mybir
    import concourse.tile as tile
    from concourse.tile_rust import add_dep_helper

    _patch_act_tables()

    dt = mybir.dt
    f32 = dt.float32
    bf16 = dt.bfloat16
    obf = bf16 if OUT_BF16 else f32
    AF = mybir.ActivationFunctionType
    ALU = mybir.AluOpType

    nc = bacc.Bacc(trn_type="TRN2")

    # ---------------- DRAM parameters ----------------
    d_xcatT = nc.dram_tensor("xcatT", [3 * ENC_IN, L], bf16, kind="ExternalInput")
    d_wflat = nc.dram_tensor("wflat", [3 * ENC_IN, D], bf16, kind="ExternalInput")
    d_pe = nc.dram_tensor("pe", [L, D], f32, kind="ExternalInput")
    d_dist2 = nc.dram_tensor("dist2", [L, L], f32, kind="ExternalInput")
    d_ident = nc.dram_tensor("ident", [P, P], f32, kind="ExternalInput")

    d_w = []
    for i in range(NLAYERS):
        lw = {}
        for nmw in ("Wq", "Wk", "Wv", "Wo", "W1", "W2"):
            lw[nmw] = nc.dram_tensor(f"{nmw}{i}", [D, D], bf16, kind="ExternalInput")
        lw["Wsig"] = nc.dram_tensor(f"Wsig{i}", [D, H], f32, kind="ExternalInput")
        lw["bq"] = nc.dram_tensor(f"bq{i}", [P, NCH], f32, kind="ExternalInput")
        lw["bk"] = nc.dram_tensor(f"bk{i}", [P, NCH], f32, kind="ExternalInput")
        lw["bf1"] = nc.dram_tensor(f"bf1{i}", [P, NCH], f32, kind="ExternalInput")
        lw["bv_bc"] = nc.dram_tensor(f"bv_bc{i}", [P, D], f32, kind="ExternalInput")
        lw["bsig_bc"] = nc.dram_tensor(f"bsig_bc{i}", [P, H], f32, kind="ExternalInput")
        for nmo in ("bo_bc", "bf2_bc"):
            if gates[nmo[:-3]]:
                lw[nmo] = nc.dram_tensor(f"{nmo}{i}", [P, D], f32, kind="ExternalInput")
        if gates["ln1"]:
            lw["g1_bc"] = nc.dram_tensor(f"g1_bc{i}", [P, D], f32, kind="ExternalInput")
            lw["b1_bc"] = nc.dram_tensor(f"b1_bc{i}", [P, D], f32, kind="ExternalInput")
        if gates["ln2"]:
            lw["g2_bc"] = nc.dram_tensor(f"g2_bc{i}", [P, D], f32, kind="ExternalInput")
            lw["b2_bc"] = nc.dram_tensor(f"b2_bc{i}", [P, D], f32, kind="ExternalInput")
        d_w.append(lw)
    d_wp = nc.dram_tensor("Wp", [D, C_OUT], bf16, kind="ExternalInput")
    if gates["lnf"]:
        d_gn = nc.dram_tensor("gn_bc", [P, D], f32, kind="ExternalInput")
        d_bn = nc.dram_tensor("bn_bc", [P, D], f32, kind="ExternalInput")
    if gates["bp"]:
        d_bp = nc.dram_tensor("bp_bc", [P, C_OUT], f32, kind="ExternalInput")

    d_out = nc.dram_tensor("out", [L, C_OUT], f32, kind="ExternalOutput")
    d_series = [
        nc.dram_tensor(f"series{i}", [H, L, L], obf, kind="ExternalOutput")
        for i in range(NLAYERS)
    ]
    d_prior = [
        nc.dram_tensor(f"prior{i}", [H, L, L], obf, kind="ExternalOutput")
        for i in range(NLAYERS)
    ]
    d_sigma = [
        nc.dram_tensor(f"sigma{i}", [H, L, L], obf, kind="ExternalOutput")
        for i in range(NLAYERS)
    ]

    def chunked(dram):  # [512, N] dram -> [128, 4, N] AP
        return dram[:, :].rearrange("(c p) n -> p c n", p=P)

    with tile.TileContext(nc) as tc:
        with (
            tc.tile_pool(name="const", bufs=1) as cpool,
            tc.tile_pool(name="wts", bufs=1) as wpool,
            tc.tile_pool(name="st1", bufs=1) as spool1,
            tc.tile_pool(name="st2", bufs=2) as spool2,
            tc.tile_pool(name="tmat", bufs=2) as tpool,
            tc.tile_pool(name="proj", bufs=1) as ppool,
            tc.tile_pool(name="ser", bufs=5 if OUT_BF16 else 3) as serpool,
            tc.tile_pool(name="serT", bufs=4) as serTpool,
            tc.tile_pool(name="outs", bufs=2) as opool,
            tc.tile_pool(name="small", bufs=2) as smpool,
            tc.tile_pool(name="ps512", bufs=4, space="PSUM") as ps512,
            tc.tile_pool(name="psT", bufs=3, space="PSUM") as psT,
            tc.tile_pool(name="psAT", bufs=1, space="PSUM") as psAT,
        ):
            ident = cpool.tile([P, P], f32)
            ones = cpool.tile([P, D], f32)
            nc.vector.memset(ones[:], 1.0)
            ones_bf = cpool.tile([P, D], bf16)
            nc.vector.memset(ones_bf[:], 1.0)
            eps_t = cpool.tile([P, 1], f32)
            nc.vector.memset(eps_t[:], EPS_LN)
            dist2 = cpool.tile([P, NCH, L], f32)
            wp_sb = cpool.tile([P, NCH, C_OUT], bf16)
            out_sb = cpool.tile([P, NCH, C_OUT], f32)
            ident_bf = cpool.tile([P, P], bf16)
            if gates["lnf"]:
                gn_sb = cpool.tile([P, D], f32)
                nc.sync.dma_start(gn_sb[:], d_gn[:, :])
                bn_sb = cpool.tile([P, D], f32)
                nc.sync.dma_start(bn_sb[:], d_bn[:, :])
            if gates["bp"]:
                bp_sb = cpool.tile([P, C_OUT], f32)
                nc.sync.dma_start(bp_sb[:], d_bp[:, :])

            def mm(out, lhsT, rhs, start, stop):
                nc.tensor.matmul(out, lhsT, rhs, start=start, stop=stop)

            # PE transpose of a [128, 4, 512] natural tile -> [128, 4, 512]
            # transposed tile (chunk index swaps roles).
            def transpose_512(dst_tile, src_tile, dst2=None, on_scalar=False):
                for n in range(NCH):
                    ps = psT.tile([P, L], f32, tag="psT")
                    for m in range(NCH):
                        nc.tensor.transpose(
                            ps[:, m * P : (m + 1) * P],
                            src_tile[:, m, n * P : (n + 1) * P],
                            ident[:],
                        )
                    if on_scalar:
                        nc.scalar.copy(dst_tile[:, n, :], ps[:])
                    else:
                        nc.vector.tensor_copy(dst_tile[:, n, :], ps[:])
                    if dst2 is not None:
                        nc.scalar.copy(dst2[:, n, :], ps[:])

            def layernorm(dst_tile, src_tile, g_bc, b_bc):
                mv = smpool.tile([P, NCH, 2], f32, tag="mv", name="mv")
                for c in range(NCH):
                    st6 = smpool.tile([P, 6], f32, tag="st6", name="st6")
                    nc.vector.bn_stats(st6[:], src_tile[:, c, :])
                    nc.vector.bn_aggr(mv[:, c, :], st6[:])
                lnv = smpool.tile([P, NCH], f32, tag="lnv", name="lnv")
                nc.scalar.activation(lnv[:], mv[:, :, 1], AF.Ln, bias=eps_t[:])
                rstd = smpool.tile([P, NCH], f32, tag="rstd", name="rstd")
                nc.scalar.activation(rstd[:], lnv[:], AF.Exp, scale=-0.5)
                for c in range(NCH):
                    nc.vector.tensor_scalar(
                        dst_tile[:, c, :],
                        src_tile[:, c, :],
                        mv[:, c, 0:1],
                        rstd[:, c : c + 1],
                        op0=ALU.subtract,
                        op1=ALU.mult,
                    )
                    if g_bc is not None:
                        nc.vector.scalar_tensor_tensor(
                            dst_tile[:, c, :], dst_tile[:, c, :], 1.0, g_bc[:],
                            op0=ALU.mult, op1=ALU.mult,
                        )
                    if b_bc is not None:
                        nc.vector.scalar_tensor_tensor(
                            dst_tile[:, c, :], dst_tile[:, c, :], 1.0, b_bc[:],
                            op0=ALU.mult, op1=ALU.add,
                        )

            # ---------------- embedding ----------------
            h_nat = spool1.tile([P, NCH, D], f32, tag="h")
            with tc.tile_pool(name="embed", bufs=1) as epool:
                xcatT = epool.tile([3 * ENC_IN, L], bf16)
                nc.sync.dma_start(xcatT[:], d_xcatT[:, :])
                wflat = epool.tile([3 * ENC_IN, D], bf16)
                nc.sync.dma_start(wflat[:], d_wflat[:, :])
                pe_sb = epool.tile([P, NCH, D], f32)
                nc.sync.dma_start(pe_sb[:], chunked(d_pe))
                for m in range(NCH):
                    ps = ps512.tile([P, D], f32, tag="ps512")
                    mm(ps[:], xcatT[:, m * P : (m + 1) * P], wflat[:], True, True)
                    nc.vector.scalar_tensor_tensor(
                        h_nat[:, m, :], ps[:], 1.0, pe_sb[:, m, :],
                        op0=ALU.mult, op1=ALU.add,
                    )

            nc.sync.dma_start(ident[:], d_ident[:, :])
            nc.vector.tensor_copy(ident_bf[:], ident[:])
            warm_ps = psT.tile([P, L], f32, tag="psT", name="warm_ps")
            for _w in range(24):
                nc.tensor.matmul(
                    warm_ps[:, 0:P], ident[:], ident[:],
                    start=True, stop=True,
                )
            nc.sync.dma_start(dist2[:], chunked(d_dist2))
            nc.sync.dma_start(wp_sb[:], d_wp[:, :].rearrange("(c p) n -> p c n", p=P))

            # ---------------- layers ----------------
            for li in range(NLAYERS):
                lw = d_w[li]
                wq = wpool.tile([P, NCH, D], bf16, tag="wq")
                nc.sync.dma_start(wq[:], chunked(lw["Wq"]))
                wk = wpool.tile([P, NCH, D], bf16, tag="wk")
                nc.sync.dma_start(wk[:], chunked(lw["Wk"]))
                wv = wpool.tile([P, NCH, D], bf16, tag="wv")
                nc.sync.dma_start(wv[:], chunked(lw["Wv"]))
                wo = wpool.tile([P, NCH, D], bf16, tag="wo")
                nc.sync.dma_start(wo[:], chunked(lw["Wo"]))
                w1 = wpool.tile([P, NCH, D], bf16, tag="w1")
                nc.sync.dma_start(w1[:], chunked(lw["W1"]))
                w2 = wpool.tile([P, NCH, D], bf16, tag="w2")
                nc.sync.dma_start(w2[:], chunked(lw["W2"]))
                wsig = wpool.tile([P, NCH, H], f32, tag="wsig")
                nc.sync.dma_start(wsig[:], chunked(lw["Wsig"]))
                bq_sb = wpool.tile([P, NCH], f32, tag="bq")
                nc.sync.dma_start(bq_sb[:], lw["bq"][:, :])
                bk_sb = wpool.tile([P, NCH], f32, tag="bk")
                nc.sync.dma_start(bk_sb[:], lw["bk"][:, :])
                bf1_sb = wpool.tile([P, NCH], f32, tag="bf1")
                nc.sync.dma_start(bf1_sb[:], lw["bf1"][:, :])
                bv_sb = wpool.tile([P, D], f32, tag="bv")
                nc.sync.dma_start(bv_sb[:], lw["bv_bc"][:, :])
                bsig_sb = wpool.tile([P, H], f32, tag="bsig")
                nc.sync.dma_start(bsig_sb[:], lw["bsig_bc"][:, :])
                opt = {}
                for nmo in ("bo_bc", "bf2_bc", "g1_bc", "b1_bc", "g2_bc", "b2_bc"):
                    if nmo in lw:
                        t = wpool.tile([P, D], f32, tag=nmo)
                        nc.sync.dma_start(t[:], lw[nmo][:, :])
                        opt[nmo] = t

                # -- transpose h -> hT --
                hT = tpool.tile([P, NCH, D], bf16, tag="tT")
                hT32 = tpool.tile([P, NCH, D], f32, tag="hT32")
                transpose_512(hT, h_nat, dst2=hT32)

                # -- projections --
                qT = ppool.tile([P, NCH, L], bf16, tag="qT")
                kT = ppool.tile([P, NCH, L], bf16, tag="kT")
                v_sb = ppool.tile([P, NCH, D], bf16, tag="v")
                for c in range(NCH):
                    ps = ps512.tile([P, L], f32, tag="ps512")
                    for kc in range(NCH):
                        mm(ps[:], wq[:, kc, c * P : (c + 1) * P], hT[:, kc, :],
                           kc == 0, kc == NCH - 1)
                    nc.scalar.activation(
                        qT[:, c, :], ps[:], AF.Identity, bias=bq_sb[:, c : c + 1]
                    )
                for c in range(NCH):
                    ps = ps512.tile([P, L], f32, tag="ps512")
                    for kc in range(NCH):
                        mm(ps[:], wk[:, kc, c * P : (c + 1) * P], hT[:, kc, :],
                           kc == 0, kc == NCH - 1)
                    nc.scalar.activation(
                        kT[:, c, :], ps[:], AF.Identity, bias=bk_sb[:, c : c + 1]
                    )
                for c in range(NCH):
                    ps = ps512.tile([P, D], f32, tag="ps512")
                    for kc in range(NCH):
                        mm(ps[:], hT[:, kc, c * P : (c + 1) * P], wv[:, kc, :],
                           kc == 0, kc == NCH - 1)
                    nc.vector.scalar_tensor_tensor(
                        v_sb[:, c, :], ps[:], 1.0, bv_sb[:],
                        op0=ALU.mult, op1=ALU.add,
                    )

                # -- sigma scalar path (batched across chunks) --
                s_val = smpool.tile([P, NCH, H], f32, tag="s_val")
                a_val = smpool.tile([P, NCH, H], f32, tag="a_val")
                lnc = smpool.tile([P, NCH, H], f32, tag="lnc")
                sg_u = smpool.tile([P, NCH, H], f32, tag="sg_u")
                for c in range(NCH):
                    ps8 = psAT.tile([P, H], f32, tag="psAT")
                    for kc in range(NCH):
                        mm(ps8[:], hT32[:, kc, c * P : (c + 1) * P], wsig[:, kc, :],
                           kc == 0, kc == NCH - 1)
                    nc.vector.scalar_tensor_tensor(
                        sg_u[:, c, :], ps8[:], 1.0, bsig_sb[:],
                        op0=ALU.mult, op1=ALU.add,
                    )
                NH = NCH * H
                # sigmoid(5u) = 1 / (1 + exp(-5u))
                t0 = smpool.tile([P, NH], f32, tag="t0")
                nc.scalar.activation(t0[:], sg_u[:], AF.Exp, scale=-5.0)
                nc.vector.tensor_scalar(t0[:], t0[:], 1.0, None, op0=ALU.add)
                t1 = smpool.tile([P, NH], f32, tag="t1")
                nc.vector.reciprocal(t1[:], t0[:])
                nc.vector.tensor_scalar(t1[:], t1[:], 1e-5, None, op0=ALU.add)
                # s = 3^t1 - 1 = exp(t1*ln3) - 1
                t2 = smpool.tile([P, NH], f32, tag="t2")
                nc.scalar.activation(t2[:], t1[:], AF.Exp, scale=LN3)
                nc.vector.tensor_scalar(
                    s_val[:].rearrange("p c h -> p (c h)"), t2[:], 1.0, None,
                    op0=ALU.subtract,
                )
                # a = -0.5 / s^2 ; lnc = -0.5*ln(2pi) - ln(s)
                t3 = smpool.tile([P, NH], f32, tag="t3")
                nc.vector.tensor_tensor(
                    t3[:], s_val[:].rearrange("p c h -> p (c h)"),
                    s_val[:].rearrange("p c h -> p (c h)"), op=ALU.mult,
                )
                t4 = smpool.tile([P, NH], f32, tag="t4")
                nc.vector.reciprocal(t4[:], t3[:])
                nc.vector.tensor_scalar(
                    a_val[:].rearrange("p c h -> p (c h)"), t4[:], -0.5, None,
                    op0=ALU.mult,
                )
                t5 = smpool.tile([P, NH], f32, tag="t5")
                nc.scalar.activation(
                    t5[:], s_val[:].rearrange("p c h -> p (c h)"), AF.Ln
                )
                nc.vector.tensor_scalar(
                    lnc[:].rearrange("p c h -> p (c h)"), t5[:], -1.0,
                    NEG_HALF_LN_2PI, op0=ALU.mult, op1=ALU.add,
                )

                # -- attention --
                rowsums = smpool.tile([P, NCH, H], f32, tag="rowsums")
                recips = smpool.tile([P, NCH, H], f32, tag="recips")
                last_prior = None
                attnT_sb = tpool.tile([P, NCH, D], bf16, tag="tT")
                psat_tiles = [None] * NCH

                def prior_sigma(m, half):
                    pr = opool.tile([P, 4, L], obf, tag="prior", name="pr")
                    sg = opool.tile([P, 4, L], obf, tag="sigma", name="sg")
                    lp = None
                    for h4 in range(4):
                        h = half * 4 + h4
                        lp = nc.scalar.activation(
                            pr[:, h4, :], dist2[:, m, :], AF.Exp,
                            scale=a_val[:, m, h : h + 1],
                            bias=lnc[:, m, h : h + 1],
                        )
                        nc.vector.tensor_scalar(
                            sg[:, h4, :], ones_bf[:, 0:L],
                            s_val[:, m, h : h + 1], None, op0=ALU.mult,
                        )
                    dstp = d_prior[li][
                        half * 4 : half * 4 + 4, m * P : (m + 1) * P, :
                    ].rearrange("h l s -> l h s")
                    nc.sync.dma_start(dstp, pr[:])
                    dsts = d_sigma[li][
                        half * 4 : half * 4 + 4, m * P : (m + 1) * P, :
                    ].rearrange("h l s -> l h s")
                    nc.sync.dma_start(dsts, sg[:])
                    return lp

                def head_transpose_attn(half, h4, nrm_tiles):
                    h = half * 4 + h4
                    serT = serTpool.tile([P, NCH, L], obf, tag="serT", name="serT")
                    for sc in range(NCH):
                        pst = psT.tile([P, L], obf, tag="psT", name="psTb")
                        for m in range(NCH):
                            nc.tensor.transpose(
                                pst[:, m * P : (m + 1) * P],
                                nrm_tiles[m][:, h4, sc * P : (sc + 1) * P],
                                ident_bf[:] if OUT_BF16 else ident[:],
                            )
                        nc.vector.tensor_copy(serT[:, sc, :], pst[:])
                    # attnT[d, l] += v[s, d] @ serT[s, l]; two heads pack a
                    # [128, 512] psum tile (column groups 0-63 / 64-127)
                    t_idx = h // 2
                    doff = (h % 2) * DH
                    if h % 2 == 0:
                        psat_tiles[t_idx] = psAT.tile(
                            [P, L], f32, tag="psAT", name="psat"
                        )
                    psat_cur = psat_tiles[t_idx]
                    for sc in range(NCH):
                        nc.tensor.matmul(
                            psat_cur[doff : doff + DH, :],
                            v_sb[:, sc, h * DH : (h + 1) * DH],
                            serT[:, sc, :],
                            start=(sc == 0), stop=(sc == NCH - 1),
                            tile_position=(0, doff),
                        )
                    if h % 2 == 1:
                        nc.vector.tensor_copy(attnT_sb[:, t_idx, :], psat_cur[:])

                for half in range(2):
                    ser_tiles = []
                    nrm_tiles = []
                    for m in range(NCH):
                        ser_tiles.append(
                            serpool.tile([P, 4, L], obf, tag="ser", name=f"ser{m}")
                        )
                        nrm_tiles.append(
                            serpool.tile([P, 4, L], obf, tag="nrm", name=f"nrm{m}")
                        )
                    prev_h4 = None
                    for h4 in range(4):
                        h = half * 4 + h4
                        hc, off = h // 2, (h % 2) * DH
                        for m in range(NCH):
                            ps = ps512.tile([P, L], f32, tag="ps512")
                            mm(ps[:], qT[off : off + DH, hc, m * P : (m + 1) * P],
                               kT[off : off + DH, hc, :], True, True)
                            nc.scalar.activation(
                                ser_tiles[m][:, h4, :], ps[:], AF.Exp, scale=0.125,
                                accum_out=rowsums[:, m, h : h + 1],
                            )
                        nc.vector.reciprocal(
                            recips[:, :, h : h + 1], rowsums[:, :, h : h + 1]
                        )
                        for m in range(NCH):
                            nc.vector.tensor_scalar(
                                nrm_tiles[m][:, h4, :], ser_tiles[m][:, h4, :],
                                recips[:, m, h : h + 1], None, op0=ALU.mult,
                            )
                        # pipeline: previous head's transpose+attn trace AFTER
                        # this head's scores/exp so PE keeps ACT fed
                        if prev_h4 is not None:
                            head_transpose_attn(half, prev_h4, nrm_tiles)
                            last_prior = prior_sigma(prev_h4, half)
                        prev_h4 = h4
                    for m in range(NCH):
                        dst = d_series[li][
                            half * 4 : half * 4 + 4, m * P : (m + 1) * P, :
                        ].rearrange("h l s -> l h s")
                        nc.sync.dma_start(dst, nrm_tiles[m][:])
                    head_transpose_attn(half, prev_h4, nrm_tiles)
                    last_prior = prior_sigma(3, half)


                # -- output projection + residual + LN1 --
                attnT = attnT_sb
                h2 = spool2.tile([P, NCH, D], f32, tag="h2")
                for m in range(NCH):
                    ps = ps512.tile([P, D], f32, tag="ps512")
                    for kc in range(NCH):
                        mm(ps[:], attnT[:, kc, m * P : (m + 1) * P], wo[:, kc, :],
                           kc == 0, kc == NCH - 1)
                    nc.vector.scalar_tensor_tensor(
                        h2[:, m, :], ps[:], 1.0, h_nat[:, m, :],
                        op0=ALU.mult, op1=ALU.add,
                    )
                    if "bo_bc" in opt:
                        nc.vector.scalar_tensor_tensor(
                            h2[:, m, :], h2[:, m, :], 1.0, opt["bo_bc"][:],
                            op0=ALU.mult, op1=ALU.add,
                        )
                x1 = spool1.tile([P, NCH, D], f32, tag="x1")
                layernorm(x1, h2, opt.get("g1_bc"), opt.get("b1_bc"))

                # -- FFN --
                x1T = tpool.tile([P, NCH, D], bf16, tag="tT")
                transpose_512(x1T, x1)
                y1g = ppool.tile([P, NCH, L], bf16, tag="qT")
                for c in range(NCH):
                    ps = ps512.tile([P, L], f32, tag="ps512")
                    for kc in range(NCH):
                        mm(ps[:], w1[:, kc, c * P : (c + 1) * P], x1T[:, kc, :],
                           kc == 0, kc == NCH - 1)
                    g_inst = nc.scalar.activation(
                        y1g[:, c, :], ps[:], AF.Gelu, bias=bf1_sb[:, c : c + 1]
                    )
                    if last_prior is not None:
                        add_dep_helper(
                            g_inst.ins, last_prior.ins, sync=False,
                            reason="group ACT table sets: priors before gelu",
                        )
                h3 = spool2.tile([P, NCH, D], f32, tag="h2")
                for m in range(NCH):
                    ps = ps512.tile([P, D], f32, tag="ps512")
                    for kc in range(NCH):
                        mm(ps[:], y1g[:, kc, m * P : (m + 1) * P], w2[:, kc, :],
                           kc == 0, kc == NCH - 1)
                    nc.vector.scalar_tensor_tensor(
                        h3[:, m, :], ps[:], 1.0, x1[:, m, :],
                        op0=ALU.mult, op1=ALU.add,
                    )
                    if "bf2_bc" in opt:
                        nc.vector.scalar_tensor_tensor(
                            h3[:, m, :], h3[:, m, :], 1.0, opt["bf2_bc"][:],
                            op0=ALU.mult, op1=ALU.add,
                        )
                h_nat = spool1.tile([P, NCH, D], f32, tag="h")
                layernorm(h_nat, h3, opt.get("g2_bc"), opt.get("b2_bc"))

            # ---------------- final norm + projection ----------------
            hf = spool1.tile([P, NCH, D], f32, tag="x1")
            layernorm(
                hf, h_nat,
                gn_sb if gates["lnf"] else None,
                bn_sb if gates["lnf"] else None,
            )
            hfT = tpool.tile([P, NCH, D], bf16, tag="tT")
            transpose_512(hfT, hf)
            for m in range(NCH):
                ps = psAT.tile([P, C_OUT], f32, tag="psAT")
                for kc in range(NCH):
                    mm(ps[:], hfT[:, kc, m * P : (m + 1) * P], wp_sb[:, kc, :],
                       kc == 0, kc == NCH - 1)
                if gates["bp"]:
                    nc.vector.scalar_tensor_tensor(
                        out_sb[:, m, :], ps[:], 1.0, bp_sb[:],
                        op0=ALU.mult, op1=ALU.add,
                    )
                else:
                    nc.vector.tensor_copy(out_sb[:, m, :], ps[:])
            nc.sync.dma_start(
                d_out[:, :].rearrange("(c p) n -> p c n", p=P), out_sb[:]
            )

    nc.finalize()
    return nc


def _np(a):
    return np.asarray(a, dtype=np.float32)


def _bf(a):
    import ml_dtypes

    return np.asarray(a, dtype=np.float32).astype(ml_dtypes.bfloat16)


def _prep_inputs(x, params):
    """Host-side layout prep: pure indexing/broadcast of inputs + constants."""
    x = _np(x)
    layers = params["layers"]

    idx = np.arange(L, dtype=np.float32)
    dist2 = np.square(np.abs(idx[:, None] - idx[None, :])).astype(np.float32)
    pos = idx[:, None]
    div = np.exp(
        np.arange(0, D, 2, dtype=np.float32) * (-math.log(10000.0) / D)
    ).astype(np.float32)
    pe = np.zeros((L, D), np.float32)
    pe[:, 0::2] = np.sin(pos * div)
    pe[:, 1::2] = np.cos(pos * div)
    ident = np.eye(P, dtype=np.float32)
    wflat = _np(params["tok_w"]).reshape(3 * ENC_IN, D)

    def percol(b):  # [512] -> [128, 4] per-partition layout
        return np.ascontiguousarray(_np(b).reshape(NCH, P).T)

    def bc(v, n):
        return np.ascontiguousarray(np.broadcast_to(_np(v), (P, n)))

    common = {
        "wflat": np.ascontiguousarray(_bf(wflat)),
        "pe": pe,
        "dist2": dist2,
        "ident": ident,
        "Wp": _bf(params["Wp"]),
    }
    gates = {}
    gates["bp"] = bool(np.any(_np(params["bp"]) != 0))
    if gates["bp"]:
        common["bp_bc"] = bc(params["bp"], C_OUT)
    gates["lnf"] = bool(
        np.any(_np(params["norm_g"]) != 1) or np.any(_np(params["norm_b"]) != 0)
    )
    if gates["lnf"]:
        common["gn_bc"] = bc(params["norm_g"], D)
        common["bn_bc"] = bc(params["norm_b"], D)
    gates["bo"] = any(np.any(_np(p["bo"]) != 0) for p in layers)
    gates["bf2"] = any(np.any(_np(p["bf2"]) != 0) for p in layers)
    gates["ln1"] = any(
        np.any(_np(p["g1"]) != 1) or np.any(_np(p["b1"]) != 0) for p in layers
    )
    gates["ln2"] = any(
        np.any(_np(p["g2"]) != 1) or np.any(_np(p["b2"]) != 0) for p in layers
    )
    for i, p in enumerate(layers):
        for nmw in ("Wq", "Wk", "Wv", "Wo", "W1", "W2"):
            common[f"{nmw}{i}"] = _bf(p[nmw])
        common[f"Wsig{i}"] = _np(p["Wsig"])
        common[f"bq{i}"] = percol(p["bq"])
        common[f"bk{i}"] = percol(p["bk"])
        common[f"bf1{i}"] = percol(p["bf1"])
        common[f"bv_bc{i}"] = bc(p["bv"], D)
        common[f"bsig_bc{i}"] = bc(p["bsig"], H)
        if gates["bo"]:
            common[f"bo_bc{i}"] = bc(p["bo"], D)
        if gates["bf2"]:
            common[f"bf2_bc{i}"] = bc(p["bf2"], D)
        if gates["ln1"]:
            common[f"g1_bc{i}"] = bc(p["g1"], D)
            common[f"b1_bc{i}"] = bc(p["b1"], D)
        if gates["ln2"]:
            common[f"g2_bc{i}"] = bc(p["g2"], D)
            common[f"b2_bc{i}"] = bc(p["b2"], D)

    in_maps = []
    for b in range(B):
        xb = x[b]
        xm1 = np.roll(xb, 1, axis=0)
        xp1 = np.roll(xb, -1, axis=0)
        xcat = np.concatenate([xm1, xb, xp1], axis=1)  # [512, 114]
        m = dict(common)
        m["xcatT"] = np.ascontiguousarray(_bf(xcat.T))
        in_maps.append(m)
    return in_maps, gates


def kernel(x, params, _trace=False):
    from concourse.bass_utils import run_bass_kernel_spmd

    in_maps, gates = _prep_inputs(x, params)
    key = tuple(sorted(gates.items()))
    if key not in _CACHE:
        _CACHE[key] = _build(gates)
    nc = _CACHE[key]

    res = run_bass_kernel_spmd(nc, in_maps, core_ids=list(range(B)), trace=_trace)

    def f32out(name):
        return np.stack(
            [np.asarray(res.results[b][name], np.float32) for b in range(B)], axis=0
        )

    out = f32out("out")
    series = tuple(f32out(f"series{i}") for i in range(NLAYERS))
    prior = tuple(f32out(f"prior{i}") for i in range(NLAYERS))
    sigma = tuple(f32out(f"sigma{i}") for i in range(NLAYERS))
    ret = (out, series, prior, sigma)
    if _trace:
        return ret, res
    return ret


# revision 34
# speedup vs baseline: 1.1681x; 1.1485x over previous
"""AnomalyTransformer forward on 8 TRN2 NeuronCores — data parallel over batch.

Self-contained: hardcodes shapes (B=8, L=512, ENC_IN=38, D=512, H=8, DFF=512,
NLAYERS=3) and shards batch b -> core b.  Returns the reference pytree
(out, (series0..2), (prior0..2), (sigma0..2)) as float32 numpy arrays.
"""

import math
import os
import sys

import numpy as np

for _p in ("/opt/trn_rl_repo", "/root/.axon_site/_ro/trn_rl_repo"):
    if os.path.isdir(_p) and _p not in sys.path:
        sys.path.insert(0, _p)
        break

B, L, ENC_IN, C_OUT, D, H, DFF, NLAYERS = 8, 512, 38, 38, 512, 8, 512, 3
P = 128
NCH = D // P          # 4 chunks of 128 along any 512 dim
DH = D // H           # 64 head dim
LN3 = math.log(3.0)
NEG_HALF_LN_2PI = -0.5 * math.log(2.0 * math.pi)
EPS_LN = 1e-5

# Output HBM dtype for the big [H,L,L] tensors: bfloat16 halves the DMA-out
# traffic (~3e-3 relative error on those outputs); set to 0 for full fp32.
OUT_BF16 = os.environ.get("KERNEL_OUT_BF16", "1") == "1"

_CACHE = {}


def _patch_act_tables():
    # The act-table-load inserter greedily picks the first set containing a
    # function, so alternating Exp/Ln thrash-loads two different sets.  Filter
    # the map it sees so Exp and Ln only resolve to the combined
    # natural_log_exp_and_others set (which really contains both).
    import concourse.bacc as bacc
    import concourse.hw_specs as hw_specs
    import concourse.mybir as mybir

    if getattr(bacc, "_act_tables_patched", False):
        return
    AF = mybir.ActivationFunctionType
    orig = bacc.get_activation_tables

    def patched(arch):
        tables = orig(arch)
        for name, fns in tables.items():
            if name != "natural_log_exp_and_others":
                fns.discard(AF.Exp)
                fns.discard(AF.Ln)
        return tables

    bacc.get_activation_tables = patched
    bacc._act_tables_patched = True


def _build(gates):
    import concourse.bacc as bacc
    import concourse.mybir as mybir
    import concourse.tile as tile
    from concourse.tile_rust import add_dep_helper

    _patch_act_tables()

    dt = mybir.dt
    f32 = dt.float32
    bf16 = dt.bfloat16
    obf = bf16 if OUT_BF16 else f32
    AF = mybir.ActivationFunctionType
    ALU = mybir.AluOpType

    nc = bacc.Bacc(trn_type="TRN2")

    # ---------------- DRAM parameters ----------------
    d_xcatT = nc.dram_tensor("xcatT", [3 * ENC_IN, L], bf16, kind="ExternalInput")
    d_wflat = nc.dram_tensor("wflat", [3 * ENC_IN, D], bf16, kind="ExternalInput")
    d_pe = nc.dram_tensor("pe", [L, D], f32, kind="ExternalInput")
    d_dist2 = nc.dram_tensor("dist2", [L, L], f32, kind="ExternalInput")
    d_ident = nc.dram_tensor("ident", [P, P], f32, kind="ExternalInput")

    d_w = []
    for i in range(NLAYERS):
        lw = {}
        for nmw in ("Wq", "Wk", "Wv", "Wo", "W1", "W2"):
            lw[nmw] = nc.dram_tensor(f"{nmw}{i}", [D, D], bf16, kind="ExternalInput")
        lw["Wsig"] = nc.dram_tensor(f"Wsig{i}", [D, H], f32, kind="ExternalInput")
        lw["bq"] = nc.dram_tensor(f"bq{i}", [P, NCH], f32, kind="ExternalInput")
        lw["bk"] = nc.dram_tensor(f"bk{i}", [P, NCH], f32, kind="ExternalInput")
        lw["bf1"] = nc.dram_tensor(f"bf1{i}", [P, NCH], f32, kind="ExternalInput")
        lw["bv_bc"] = nc.dram_tensor(f"bv_bc{i}", [P, D], f32, kind="ExternalInput")
        lw["bsig_bc"] = nc.dram_tensor(f"bsig_bc{i}", [P, H], f32, kind="ExternalInput")
        for nmo in ("bo_bc", "bf2_bc"):
            if gates[nmo[:-3]]:
                lw[nmo] = nc.dram_tensor(f"{nmo}{i}", [P, D], f32, kind="ExternalInput")
        if gates["ln1"]:
            lw["g1_bc"] = nc.dram_tensor(f"g1_bc{i}", [P, D], f32, kind="ExternalInput")
            lw["b1_bc"] = nc.dram_tensor(f"b1_bc{i}", [P, D], f32, kind="ExternalInput")
        if gates["ln2"]:
            lw["g2_bc"] = nc.dram_tensor(f"g2_bc{i}", [P, D], f32, kind="ExternalInput")
            lw["b2_bc"] = nc.dram_tensor(f"b2_bc{i}", [P, D], f32, kind="ExternalInput")
        d_w.append(lw)
    d_wp = nc.dram_tensor("Wp", [D, C_OUT], bf16, kind="ExternalInput")
    if gates["lnf"]:
        d_gn = nc.dram_tensor("gn_bc", [P, D], f32, kind="ExternalInput")
        d_bn = nc.dram_tensor("bn_bc", [P, D], f32, kind="ExternalInput")
    if gates["bp"]:
        d_bp = nc.dram_tensor("bp_bc", [P, C_OUT], f32, kind="ExternalInput")

    d_out = nc.dram_tensor("out", [L, C_OUT], f32, kind="ExternalOutput")
    d_series = [
        nc.dram_tensor(f"series{i}", [H, L, L], obf, kind="ExternalOutput")
        for i in range(NLAYERS)
    ]
    d_prior = [
        nc.dram_tensor(f"prior{i}", [H, L, L], obf, kind="ExternalOutput")
        for i in range(NLAYERS)
    ]
    d_sigma = [
        nc.dram_tensor(f"sigma{i}", [H, L, L], obf, kind="ExternalOutput")
        for i in range(NLAYERS)
    ]

    def chunked(dram):  # [512, N] dram -> [128, 4, N] AP
        return dram[:, :].rearrange("(c p) n -> p c n", p=P)

    with tile.TileContext(nc) as tc:
        with (
            tc.tile_pool(name="const", bufs=1) as cpool,
            tc.tile_pool(name="wts", bufs=1) as wpool,
            tc.tile_pool(name="st1", bufs=1) as spool1,
            tc.tile_pool(name="st2", bufs=2) as spool2,
            tc.tile_pool(name="tmat", bufs=2) as tpool,
            tc.tile_pool(name="proj", bufs=1) as ppool,
            tc.tile_pool(name="ser", bufs=5 if OUT_BF16 else 3) as serpool,
            tc.tile_pool(name="serT", bufs=4) as serTpool,
            tc.tile_pool(name="outs", bufs=2) as opool,
            tc.tile_pool(name="small", bufs=2) as smpool,
            tc.tile_pool(name="ps512", bufs=4, space="PSUM") as ps512,
            tc.tile_pool(name="psT", bufs=3, space="PSUM") as psT,
            tc.tile_pool(name="psAT", bufs=1, space="PSUM") as psAT,
        ):
            ident = cpool.tile([P, P], f32)
            ones = cpool.tile([P, D], f32)
            nc.vector.memset(ones[:], 1.0)
            ones_bf = cpool.tile([P, D], bf16)
            nc.vector.memset(ones_bf[:], 1.0)
            eps_t = cpool.tile([P, 1], f32)
            nc.vector.memset(eps_t[:], EPS_LN)
            dist2 = cpool.tile([P, NCH, L], f32)
            wp_sb = cpool.tile([P, NCH, C_OUT], bf16)
            out_sb = cpool.tile([P, NCH, C_OUT], f32)
            ident_bf = cpool.tile([P, P], bf16)
            if gates["lnf"]:
                gn_sb = cpool.tile([P, D], f32)
                nc.sync.dma_start(gn_sb[:], d_gn[:, :])
                bn_sb = cpool.tile([P, D], f32)
                nc.sync.dma_start(bn_sb[:], d_bn[:, :])
            if gates["bp"]:
                bp_sb = cpool.tile([P, C_OUT], f32)
                nc.sync.dma_start(bp_sb[:], d_bp[:, :])

            def mm(out, lhsT, rhs, start, stop):
                nc.tensor.matmul(out, lhsT, rhs, start=start, stop=stop)

            # PE transpose of a [128, 4, 512] natural tile -> [128, 4, 512]
            # transposed tile (chunk index swaps roles).
            def transpose_512(dst_tile, src_tile, dst2=None, on_scalar=False):
                for n in range(NCH):
                    ps = psT.tile([P, L], f32, tag="psT")
                    for m in range(NCH):
                        nc.tensor.transpose(
                            ps[:, m * P : (m + 1) * P],
                            src_tile[:, m, n * P : (n + 1) * P],
                            ident[:],
                        )
                    if on_scalar:
                        nc.scalar.copy(dst_tile[:, n, :], ps[:])
                    else:
                        nc.vector.tensor_copy(dst_tile[:, n, :], ps[:])
                    if dst2 is not None:
                        nc.scalar.copy(dst2[:, n, :], ps[:])

            def layernorm(dst_tile, src_tile, g_bc, b_bc):
                mv = smpool.tile([P, NCH, 2], f32, tag="mv", name="mv")
                for c in range(NCH):
                    st6 = smpool.tile([P, 6], f32, tag="st6", name="st6")
                    nc.vector.bn_stats(st6[:], src_tile[:, c, :])
                    nc.vector.bn_aggr(mv[:, c, :], st6[:])
                lnv = smpool.tile([P, NCH], f32, tag="lnv", name="lnv")
                nc.scalar.activation(lnv[:], mv[:, :, 1], AF.Ln, bias=eps_t[:])
                rstd = smpool.tile([P, NCH], f32, tag="rstd", name="rstd")
                nc.scalar.activation(rstd[:], lnv[:], AF.Exp, scale=-0.5)
                for c in range(NCH):
                    nc.vector.tensor_scalar(
                        dst_tile[:, c, :],
                        src_tile[:, c, :],
                        mv[:, c, 0:1],
                        rstd[:, c : c + 1],
                        op0=ALU.subtract,
                        op1=ALU.mult,
                    )
                    if g_bc is not None:
                        nc.vector.scalar_tensor_tensor(
                            dst_tile[:, c, :], dst_tile[:, c, :], 1.0, g_bc[:],
                            op0=ALU.mult, op1=ALU.mult,
                        )
                    if b_bc is not None:
                        nc.vector.scalar_tensor_tensor(
                            dst_tile[:, c, :], dst_tile[:, c, :], 1.0, b_bc[:],
                            op0=ALU.mult, op1=ALU.add,
                        )

            # ---------------- embedding ----------------
            h_nat = spool1.tile([P, NCH, D], f32, tag="h")
            with tc.tile_pool(name="embed", bufs=1) as epool:
                xcatT = epool.tile([3 * ENC_IN, L], bf16)
                nc.sync.dma_start(xcatT[:], d_xcatT[:, :])
                wflat = epool.tile([3 * ENC_IN, D], bf16)
                nc.sync.dma_start(wflat[:], d_wflat[:, :])
                pe_sb = epool.tile([P, NCH, D], f32)
                nc.sync.dma_start(pe_sb[:], chunked(d_pe))
                for m in range(NCH):
                    ps = ps512.tile([P, D], f32, tag="ps512")
                    mm(ps[:], xcatT[:, m * P : (m + 1) * P], wflat[:], True, True)
                    nc.vector.scalar_tensor_tensor(
                        h_nat[:, m, :], ps[:], 1.0, pe_sb[:, m, :],
                        op0=ALU.mult, op1=ALU.add,
                    )

            nc.sync.dma_start(ident[:], d_ident[:, :])
            nc.vector.tensor_copy(ident_bf[:], ident[:])
            warm_ps = psT.tile([P, L], f32, tag="psT", name="warm_ps")
            for _w in range(24):
                nc.tensor.matmul(
                    warm_ps[:, 0:P], ident[:], ident[:],
                    start=True, stop=True,
                )
            nc.sync.dma_start(dist2[:], chunked(d_dist2))
            nc.sync.dma_start(wp_sb[:], d_wp[:, :].rearrange("(c p) n -> p c n", p=P))

            # ---------------- layers ----------------
            for li in range(NLAYERS):
                lw = d_w[li]
                wq = wpool.tile([P, NCH, D], bf16, tag="wq")
                nc.sync.dma_start(wq[:], chunked(lw["Wq"]))
                wk = wpool.tile([P, NCH, D], bf16, tag="wk")
                nc.sync.dma_start(wk[:], chunked(lw["Wk"]))
                wv = wpool.tile([P, NCH, D], bf16, tag="wv")
                nc.sync.dma_start(wv[:], chunked(lw["Wv"]))
                wo = wpool.tile([P, NCH, D], bf16, tag="wo")
                nc.sync.dma_start(wo[:], chunked(lw["Wo"]))
                w1 = wpool.tile([P, NCH, D], bf16, tag="w1")
                nc.sync.dma_start(w1[:], chunked(lw["W1"]))
                w2 = wpool.tile([P, NCH, D], bf16, tag="w2")
                nc.sync.dma_start(w2[:], chunked(lw["W2"]))
                wsig = wpool.tile([P, NCH, H], f32, tag="wsig")
                nc.sync.dma_start(wsig[:], chunked(lw["Wsig"]))
                bq_sb = wpool.tile([P, NCH], f32, tag="bq")
                nc.sync.dma_start(bq_sb[:], lw["bq"][:, :])
                bk_sb = wpool.tile([P, NCH], f32, tag="bk")
                nc.sync.dma_start(bk_sb[:], lw["bk"][:, :])
                bf1_sb = wpool.tile([P, NCH], f32, tag="bf1")
                nc.sync.dma_start(bf1_sb[:], lw["bf1"][:, :])
                bv_sb = wpool.tile([P, D], f32, tag="bv")
                nc.sync.dma_start(bv_sb[:], lw["bv_bc"][:, :])
                bsig_sb = wpool.tile([P, H], f32, tag="bsig")
                nc.sync.dma_start(bsig_sb[:], lw["bsig_bc"][:, :])
                opt = {}
                for nmo in ("bo_bc", "bf2_bc", "g1_bc", "b1_bc", "g2_bc", "b2_bc"):
                    if nmo in lw:
                        t = wpool.tile([P, D], f32, tag=nmo)
                        nc.sync.dma_start(t[:], lw[nmo][:, :])
                        opt[nmo] = t

                # -- transpose h -> hT --
                hT = tpool.tile([P, NCH, D], bf16, tag="tT")
                hT32 = tpool.tile([P, NCH, D], f32, tag="hT32")
                transpose_512(hT, h_nat, dst2=hT32)

                # -- projections --
                qT = ppool.tile([P, NCH, L], bf16, tag="qT")
                kT = ppool.tile([P, NCH, L], bf16, tag="kT")
                v_sb = ppool.tile([P, NCH, D], bf16, tag="v")
                for c in range(NCH):
                    ps = ps512.tile([P, L], f32, tag="ps512")
                    for kc in range(NCH):
                        mm(ps[:], wq[:, kc, c * P : (c + 1) * P], hT[:, kc, :],
                           kc == 0, kc == NCH - 1)
                    nc.scalar.activation(
                        qT[:, c, :], ps[:], AF.Identity, bias=bq_sb[:, c : c + 1]
                    )
                for c in range(NCH):
                    ps = ps512.tile([P, L], f32, tag="ps512")
                    for kc in range(NCH):
                        mm(ps[:], wk[:, kc, c * P : (c + 1) * P], hT[:, kc, :],
                           kc == 0, kc == NCH - 1)
                    nc.scalar.activation(
                        kT[:, c, :], ps[:], AF.Identity, bias=bk_sb[:, c : c + 1]
                    )
                for c in range(NCH):
                    ps = ps512.tile([P, D], f32, tag="ps512")
                    for kc in range(NCH):
                        mm(ps[:], hT[:, kc, c * P : (c + 1) * P], wv[:, kc, :],
                           kc == 0, kc == NCH - 1)
                    nc.vector.scalar_tensor_tensor(
                        v_sb[:, c, :], ps[:], 1.0, bv_sb[:],
                        op0=ALU.mult, op1=ALU.add,
                    )

                # -- sigma scalar path (batched across chunks) --
                s_val = smpool.tile([P, NCH, H], f32, tag="s_val")
                a_val = smpool.tile([P, NCH, H], f32, tag="a_val")
                lnc = smpool.tile([P, NCH, H], f32, tag="lnc")
                sg_u = smpool.tile([P, NCH, H], f32, tag="sg_u")
                for c in range(NCH):
                    ps8 = psAT.tile([P, H], f32, tag="psAT")
                    for kc in range(NCH):
                        mm(ps8[:], hT32[:, kc, c * P : (c + 1) * P], wsig[:, kc, :],
                           kc == 0, kc == NCH - 1)
                    nc.vector.scalar_tensor_tensor(
                        sg_u[:, c, :], ps8[:], 1.0, bsig_sb[:],
                        op0=ALU.mult, op1=ALU.add,
                    )
                NH = NCH * H
                # sigmoid(5u) = 1 / (1 + exp(-5u))
                t0 = smpool.tile([P, NH], f32, tag="t0")
                nc.scalar.activation(t0[:], sg_u[:], AF.Exp, scale=-5.0)
                nc.vector.tensor_scalar(t0[:], t0[:], 1.0, None, op0=ALU.add)
                t1 = smpool.tile([P, NH], f32, tag="t1")
                nc.vector.reciprocal(t1[:], t0[:])
                nc.vector.tensor_scalar(t1[:], t1[:], 1e-5, None, op0=ALU.add)
                # s = 3^t1 - 1 = exp(t1*ln3) - 1
                t2 = smpool.tile([P, NH], f32, tag="t2")
                nc.scalar.activation(t2[:], t1[:], AF.Exp, scale=LN3)
                nc.vector.tensor_scalar(
                    s_val[:].rearrange("p c h -> p (c h)"), t2[:], 1.0, None,
                    op0=ALU.subtract,
                )
                # a = -0.5 / s^2 ; lnc = -0.5*ln(2pi) - ln(s)
                t3 = smpool.tile([P, NH], f32, tag="t3")
                nc.vector.tensor_tensor(
                    t3[:], s_val[:].rearrange("p c h -> p (c h)"),
                    s_val[:].rearrange("p c h -> p (c h)"), op=ALU.mult,
                )
                t4 = smpool.tile([P, NH], f32, tag="t4")
                nc.vector.reciprocal(t4[:], t3[:])
                nc.vector.tensor_scalar(
                    a_val[:].rearrange("p c h -> p (c h)"), t4[:], -0.5, None,
                    op0=ALU.mult,
                )
                t5 = smpool.tile([P, NH], f32, tag="t5")
                nc.scalar.activation(
                    t5[:], s_val[:].rearrange("p c h -> p (c h)"), AF.Ln
                )
                nc.vector.tensor_scalar(
                    lnc[:].rearrange("p c h -> p (c h)"), t5[:], -1.0,
                    NEG_HALF_LN_2PI, op0=ALU.mult, op1=ALU.add,
                )

                # -- attention --
                rowsums = smpool.tile([P, NCH, H], f32, tag="rowsums")
                recips = smpool.tile([P, NCH, H], f32, tag="recips")
                last_prior = None
                attnT_sb = tpool.tile([P, NCH, D], bf16, tag="tT")
                psat_tiles = [None] * NCH

                def prior_sigma(m, half):
                    pr = opool.tile([P, 4, L], obf, tag="prior", name="pr")
                    sg = opool.tile([P, 4, L], obf, tag="sigma", name="sg")
                    lp = None
                    for h4 in range(4):
                        h = half * 4 + h4
                        lp = nc.scalar.activation(
                            pr[:, h4, :], dist2[:, m, :], AF.Exp,
                            scale=a_val[:, m, h : h + 1],
                            bias=lnc[:, m, h : h + 1],
                        )
                        nc.vector.tensor_scalar(
                            sg[:, h4, :], ones_bf[:, 0:L],
                            s_val[:, m, h : h + 1], None, op0=ALU.mult,
                        )
                    dstp = d_prior[li][
                        half * 4 : half * 4 + 4, m * P : (m + 1) * P, :
                    ].rearrange("h l s -> l h s")
                    nc.sync.dma_start(dstp, pr[:])
                    dsts = d_sigma[li][
                        half * 4 : half * 4 + 4, m * P : (m + 1) * P, :
                    ].rearrange("h l s -> l h s")
                    nc.sync.dma_start(dsts, sg[:])
                    return lp

                def head_transpose_attn(half, h4, nrm_tiles):
                    h = half * 4 + h4
                    serT = serTpool.tile([P, NCH, L], obf, tag="serT", name="serT")
                    for sc in range(NCH):
                        pst = psT.tile([P, L], obf, tag="psT", name="psTb")
                        for m in range(NCH):
                            nc.tensor.transpose(
                                pst[:, m * P : (m + 1) * P],
                                nrm_tiles[m][:, h4, sc * P : (sc + 1) * P],
                                ident_bf[:] if OUT_BF16 else ident[:],
                            )
                        nc.vector.tensor_copy(serT[:, sc, :], pst[:])
                    # attnT[d, l] += v[s, d] @ serT[s, l]; two heads pack a
                    # [128, 512] psum tile (column groups 0-63 / 64-127)
                    t_idx = h // 2
                    doff = (h % 2) * DH
                    if h % 2 == 0:
                        psat_tiles[t_idx] = psAT.tile(
                            [P, L], f32, tag="psAT", name="psat"
                        )
                    psat_cur = psat_tiles[t_idx]
                    for sc in range(NCH):
                        nc.tensor.matmul(
                            psat_cur[doff : doff + DH, :],
                            v_sb[:, sc, h * DH : (h + 1) * DH],
                            serT[:, sc, :],
                            start=(sc == 0), stop=(sc == NCH - 1),
                            tile_position=(0, doff),
                        )
                    if h % 2 == 1:
                        nc.vector.tensor_copy(attnT_sb[:, t_idx, :], psat_cur[:])

                for half in range(2):
                    ser_tiles = []
                    nrm_tiles = []
                    for m in range(NCH):
                        ser_tiles.append(
                            serpool.tile([P, 4, L], obf, tag="ser", name=f"ser{m}")
                        )
                        nrm_tiles.append(
                            serpool.tile([P, 4, L], obf, tag="nrm", name=f"nrm{m}")
                        )
                    prev_h4 = None
                    for h4 in range(4):
                        h = half * 4 + h4
                        hc, off = h // 2, (h % 2) * DH
                        for m in range(NCH):
                            ps = ps512.tile([P, L], f32, tag="ps512")
                            mm(ps[:], qT[off : off + DH, hc, m * P : (m + 1) * P],
                               kT[off : off + DH, hc, :], True, True)
                            nc.scalar.activation(
                                ser_tiles[m][:, h4, :], ps[:], AF.Exp, scale=0.125,
                                accum_out=rowsums[:, m, h : h + 1],
                            )
                        nc.vector.reciprocal(
                            recips[:, :, h : h + 1], rowsums[:, :, h : h + 1]
                        )
                        for m in range(NCH):
                            nc.vector.tensor_scalar(
                                nrm_tiles[m][:, h4, :], ser_tiles[m][:, h4, :],
                                recips[:, m, h : h + 1], None, op0=ALU.mult,
                            )
                        # pipeline: previous head's transpose+attn trace AFTER
                        # this head's scores/exp so PE keeps ACT fed
                        if prev_h4 is not None:
                            head_transpose_attn(half, prev_h4, nrm_tiles)
                        prev_h4 = h4
                    for m in range(NCH):
                        dst = d_series[li][
                            half * 4 : half * 4 + 4, m * P : (m + 1) * P, :
                        ].rearrange("h l s -> l h s")
                        nc.sync.dma_start(dst, nrm_tiles[m][:])
                    head_transpose_attn(half, prev_h4, nrm_tiles)
                    for m in range(NCH):
                        last_prior = prior_sigma(m, half)


                # -- output projection + residual + LN1 --
                attnT = attnT_sb
                h2 = spool2.tile([P, NCH, D], f32, tag="h2")
                for m in range(NCH):
                    ps = ps512.tile([P, D], f32, tag="ps512")
                    for kc in range(NCH):
                        mm(ps[:], attnT[:, kc, m * P : (m + 1) * P], wo[:, kc, :],
                           kc == 0, kc == NCH - 1)
                    nc.vector.scalar_tensor_tensor(
                        h2[:, m, :], ps[:], 1.0, h_nat[:, m, :],
                        op0=ALU.mult, op1=ALU.add,
                    )
                    if "bo_bc" in opt:
                        nc.vector.scalar_tensor_tensor(
                            h2[:, m, :], h2[:, m, :], 1.0, opt["bo_bc"][:],
                            op0=ALU.mult, op1=ALU.add,
                        )
                x1 = spool1.tile([P, NCH, D], f32, tag="x1")
                layernorm(x1, h2, opt.get("g1_bc"), opt.get("b1_bc"))

                # -- FFN --
                x1T = tpool.tile([P, NCH, D], bf16, tag="tT")
                transpose_512(x1T, x1)
                y1g = ppool.tile([P, NCH, L], bf16, tag="qT")
                for c in range(NCH):
                    ps = ps512.tile([P, L], f32, tag="ps512")
                    for kc in range(NCH):
                        mm(ps[:], w1[:, kc, c * P : (c + 1) * P], x1T[:, kc, :],
                           kc == 0, kc == NCH - 1)
                    g_inst = nc.scalar.activation(
                        y1g[:, c, :], ps[:], AF.Gelu, bias=bf1_sb[:, c : c + 1]
                    )
                    if last_prior is not None:
                        add_dep_helper(
                            g_inst.ins, last_prior.ins, sync=False,
                            reason="group ACT table sets: priors before gelu",
                        )
                h3 = spool2.tile([P, NCH, D], f32, tag="h2")
                for m in range(NCH):
                    ps = ps512.tile([P, D], f32, tag="ps512")
                    for kc in range(NCH):
                        mm(ps[:], y1g[:, kc, m * P : (m + 1) * P], w2[:, kc, :],
                           kc == 0, kc == NCH - 1)
                    nc.vector.scalar_tensor_tensor(
                        h3[:, m, :], ps[:], 1.0, x1[:, m, :],
                        op0=ALU.mult, op1=ALU.add,
                    )
                    if "bf2_bc" in opt:
                        nc.vector.scalar_tensor_tensor(
                            h3[:, m, :], h3[:, m, :], 1.0, opt["bf2_bc"][:],
                            op0=ALU.mult, op1=ALU.add,
                        )
                h_nat = spool1.tile([P, NCH, D], f32, tag="h")
                layernorm(h_nat, h3, opt.get("g2_bc"), opt.get("b2_bc"))

            # ---------------- final norm + projection ----------------
            hf = spool1.tile([P, NCH, D], f32, tag="x1")
            layernorm(
                hf, h_nat,
                gn_sb if gates["lnf"] else None,
                bn_sb if gates["lnf"] else None,
            )
            hfT = tpool.tile([P, NCH, D], bf16, tag="tT")
            transpose_512(hfT, hf)
            for m in range(NCH):
                ps = psAT.tile([P, C_OUT], f32, tag="psAT")
                for kc in range(NCH):
                    mm(ps[:], hfT[:, kc, m * P : (m + 1) * P], wp_sb[:, kc, :],
                       kc == 0, kc == NCH - 1)
                if gates["bp"]:
                    nc.vector.scalar_tensor_tensor(
                        out_sb[:, m, :], ps[:], 1.0, bp_sb[:],
                        op0=ALU.mult, op1=ALU.add,
                    )
                else:
                    nc.vector.tensor_copy(out_sb[:, m, :], ps[:])
            nc.sync.dma_start(
                d_out[:, :].rearrange("(c p) n -> p c n", p=P), out_sb[:]
            )

    nc.finalize()
    return nc


def _np(a):
    return np.asarray(a, dtype=np.float32)


def _bf(a):
    import ml_dtypes

    return np.asarray(a, dtype=np.float32).astype(ml_dtypes.bfloat16)


def _prep_inputs(x, params):
    """Host-side layout prep: pure indexing/broadcast of inputs + constants."""
    x = _np(x)
    layers = params["layers"]

    idx = np.arange(L, dtype=np.float32)
    dist2 = np.square(np.abs(idx[:, None] - idx[None, :])).astype(np.float32)
    pos = idx[:, None]
    div = np.exp(
        np.arange(0, D, 2, dtype=np.float32) * (-math.log(10000.0) / D)
    ).astype(np.float32)
    pe = np.zeros((L, D), np.float32)
    pe[:, 0::2] = np.sin(pos * div)
    pe[:, 1::2] = np.cos(pos * div)
    ident = np.eye(P, dtype=np.float32)
    wflat = _np(params["tok_w"]).reshape(3 * ENC_IN, D)

    def percol(b):  # [512] -> [128, 4] per-partition layout
        return np.ascontiguousarray(_np(b).reshape(NCH, P).T)

    def bc(v, n):
        return np.ascontiguousarray(np.broadcast_to(_np(v), (P, n)))

    common = {
        "wflat": np.ascontiguousarray(_bf(wflat)),
        "pe": pe,
        "dist2": dist2,
        "ident": ident,
        "Wp": _bf(params["Wp"]),
    }
    gates = {}
    gates["bp"] = bool(np.any(_np(params["bp"]) != 0))
    if gates["bp"]:
        common["bp_bc"] = bc(params["bp"], C_OUT)
    gates["lnf"] = bool(
        np.any(_np(params["norm_g"]) != 1) or np.any(_np(params["norm_b"]) != 0)
    )
    if gates["lnf"]:
        common["gn_bc"] = bc(params["norm_g"], D)
        common["bn_bc"] = bc(params["norm_b"], D)
    gates["bo"] = any(np.any(_np(p["bo"]) != 0) for p in layers)
    gates["bf2"] = any(np.any(_np(p["bf2"]) != 0) for p in layers)
    gates["ln1"] = any(
        np.any(_np(p["g1"]) != 1) or np.any(_np(p["b1"]) != 0) for p in layers
    )
    gates["ln2"] = any(
        np.any(_np(p["g2"]) != 1) or np.any(_np(p["b2"]) != 0) for p in layers
    )
    for i, p in enumerate(layers):
        for nmw in ("Wq", "Wk", "Wv", "Wo", "W1", "W2"):
            common[f"{nmw}{i}"] = _bf(p[nmw])
        common[f"Wsig{i}"] = _np(p["Wsig"])
        common[f"bq{i}"] = percol(p["bq"])
        common[f"bk{i}"] = percol(p["bk"])
        common[f"bf1{i}"] = percol(p["bf1"])
        common[f"bv_bc{i}"] = bc(p["bv"], D)
        common[f"bsig_bc{i}"] = bc(p["bsig"], H)
        if gates["bo"]:
            common[f"bo_bc{i}"] = bc(p["bo"], D)
        if gates["bf2"]:
            common[f"bf2_bc{i}"] = bc(p["bf2"], D)
        if gates["ln1"]:
            common[f"g1_bc{i}"] = bc(p["g1"], D)
            common[f"b1_bc{i}"] = bc(p["b1"], D)
        if gates["ln2"]:
            common[f"g2_bc{i}"] = bc(p["g2"], D)
            common[f"b2_bc{i}"] = bc(p["b2"], D)

    in_maps = []
    for b in range(B):
        xb = x[b]
        xm1 = np.roll(xb, 1, axis=0)
        xp1 = np.roll(xb, -1, axis=0)
        xcat = np.concatenate([xm1, xb, xp1], axis=1)  # [512, 114]
        m = dict(common)
        m["xcatT"] = np.ascontiguousarray(_bf(xcat.T))
        in_maps.append(m)
    return in_maps, gates


def kernel(x, params, _trace=False):
    from concourse.bass_utils import run_bass_kernel_spmd

    in_maps, gates = _prep_inputs(x, params)
    key = tuple(sorted(gates.items()))
    if key not in _CACHE:
        _CACHE[key] = _build(gates)
    nc = _CACHE[key]

    res = run_bass_kernel_spmd(nc, in_maps, core_ids=list(range(B)), trace=_trace)

    def f32out(name):
        return np.stack(
            [np.asarray(res.results[b][name], np.float32) for b in range(B)], axis=0
        )

    out = f32out("out")
    series = tuple(f32out(f"series{i}") for i in range(NLAYERS))
    prior = tuple(f32out(f"prior{i}") for i in range(NLAYERS))
    sigma = tuple(f32out(f"sigma{i}") for i in range(NLAYERS))
    ret = (out, series, prior, sigma)
    if _trace:
        return ret, res
    return ret
